# revision 18
# baseline (speedup 1.0000x reference)
"""Trainium2 Bass kernel for nn_KrabbyPatty: batched NMF with MLP bread.

Per-core program (pure data parallel, one batch element per core):
  X  = relu(Xin @ W1 + b1)                  # [4096, 1024]
  D, C = D_init, C_init
  repeat 6x:
    C = C * (D^T X) / (D^T D C + eps)
    D = D * (X C^T) / (D C C^T + eps)
  out = D @ (C @ W2) + b2

Key layout/engine choices:
  - Host prep (layout only): Xin transposed + cast bf16; weights cast bf16
    and chunk-major; D/C initial states pre-arranged in device layouts.
  - XT = X^T [dout-part, l] computed by dense bf16 matmuls (PE), relu+bias
    fused on ScalarE during PSUM->SBUF eviction.
  - XB = X natural built from XT with DMA x-bar transposes (bf16,
    SBUF->SBUF) - zero PE/DVE cost.
  - D state in "dt4" layout [128=(4 l-quarters x 32r), 1024 l'] = folded
    D^T; C state in "c4c" layout [128=(4 d-quarters x 32r), 256 d'].
    All elementwise updates run on full 128 partitions.
  - D^T X and X C^T use 4x column-tiled matmuls (tile_position=(0,32b)):
    four concurrent rhs streams -> ~4x fewer PE cycles per X pass, and
    the quarter-partials land directly in the dt4/c4c layouts (the
    "reduction" is PSUM accumulation - no cross-partition sums needed).
  - D C C^T and DtD C use diagonal 32x32 tiles (tile_position=(32a,32a)).
  - Division via Ln/Exp on ScalarE (nc.scalar Reciprocal is banned for
    accuracy); numerator products on DVE with PSUM in-place reuse.
  - dnat (D natural, lhsT for D^T X) and CT (C^T chunks, lhsT for C X^T)
    rebuilt each step with one small DMA transpose each.
  - Final: C2 = C@W2 (8k accum), out tiles = D@C2 with K=32 row-tiled
    matmuls interleaved across the 4 row groups; b2 added on DVE during
    PSUM eviction, hidden behind the output DMA.
"""

import sys
import numpy as np

L, B, DM, R, K_STEPS = 4096, 8, 1024, 32, 6
EPS = 1e-9
NL = L // 128   # 32 l-tiles
ND = DM // 128  # 8 d-chunks
NQ = 4          # l-quarters (1024 each)


def build_nc():
    import concourse.bacc as bacc
    import concourse.mybir as mybir
    import concourse.tile as tile

    f32 = mybir.dt.float32
    bf16 = mybir.dt.bfloat16
    AF = mybir.ActivationFunctionType
    ALU = mybir.AluOpType

    class _Bacc(bacc.Bacc):
        """Pin all activations to the one act-func table that contains
        every function this kernel uses (relu/ln/exp/copy/identity).

        The default per-instruction set choice alternates between the
        `natural_log` and `exp_and_others` tables, inserting a ~1.3us
        ACT_TABLE_LOAD on every Ln<->Exp<->Copy transition (4+ per NMF
        step).  `natural_log_exp_and_others` holds all of them at once;
        hiding my funcs from the other sets (list order/length kept, so
        act_func_set_id indices still match act_info.json) makes the
        fixpoint hoist a single load for the whole kernel.
        """

        _KEEP = "natural_log_exp_and_others"

        def insert_act_table_loads(self):
            from concourse.hw_specs import get_activation_tables
            import bass_rust as _br

            has_activation = any(
                isinstance(i, mybir.InstActivation)
                for b in self.main_func.blocks
                for i in b.instructions
            )
            if not has_activation:
                return
            used = {
                i.func
                for b in self.main_func.blocks
                for i in b.instructions
                if isinstance(i, mybir.InstActivation)
            }
            tables = list(get_activation_tables(self.m.arch).items())
            keep_funcs = dict(tables)[self._KEEP]
            if used <= keep_funcs:
                tables = [
                    (name, funcs if name == self._KEEP else funcs - used)
                    for name, funcs in tables
                ]
            _br.insert_act_table_loads(self, tables)

    nc = _Bacc()
    xt_in = nc.dram_tensor("xt_in", [DM, L], bf16, kind="ExternalInput")
    w1s_in = nc.dram_tensor("w1s", [128, ND, DM], bf16, kind="ExternalInput")
    w2s_in = nc.dram_tensor("w2s", [128, ND, DM], bf16, kind="ExternalInput")
    b1s_in = nc.dram_tensor("b1s", [128, ND], f32, kind="ExternalInput")
    b2f_in = nc.dram_tensor("b2f", [128, DM], f32, kind="ExternalInput")
    dt4_in = nc.dram_tensor("dt4", [128, L // 4], f32, kind="ExternalInput")
    dt4b_in = nc.dram_tensor("dt4b", [128, L // 4], bf16, kind="ExternalInput")
    dnat_in = nc.dram_tensor("dnat0", [128, ND, 128], bf16, kind="ExternalInput")
    c4c_in = nc.dram_tensor("c4c", [128, DM // 4], f32, kind="ExternalInput")
    c4cb_in = nc.dram_tensor("c4cb", [128, DM // 4], bf16, kind="ExternalInput")
    out = nc.dram_tensor("out", [L, DM], f32, kind="ExternalOutput")

    with tile.TileContext(nc) as tc:
        with (
            tc.tile_pool(name="bigsb", bufs=1) as bigsb,    # xt / xb
            tc.tile_pool(name="wpool", bufs=1) as wpool,    # w1s then w2s
            tc.tile_pool(name="xq", bufs=2) as xqp,         # Xin^T quarter staging
            tc.tile_pool(name="state", bufs=1) as st,       # D/C state + consts
            tc.tile_pool(name="work", bufs=1) as wk,        # per-step recompute
            tc.tile_pool(name="big", bufs=2, space="PSUM") as psbig,    # [128,1024]
            tc.tile_pool(name="small", bufs=4, space="PSUM") as pssm,   # [128,256]
        ):
            # ---------------- constants / initial state ----------------
            b1s = st.tile([128, ND], f32, tag="b1s")
            nc.sync.dma_start(b1s[:], b1s_in[:, :])
            eps_c = st.tile([128, 1], f32, tag="epsc")
            nc.vector.memset(eps_c[:], EPS)

            # ---------------- phase 1: XT = relu(W1^T Xin^T + b1) ------
            # xt[p, j, l] = X[l, 128j + p];  xb[p, i, d] = X[128i + p, d]
            xt = bigsb.tile([128, ND, L], bf16, tag="xt")
            xb = bigsb.tile([128, NL, DM], bf16, tag="xb")
            w1s = wpool.tile([128, ND, DM], bf16, tag="wts")

            for q in range(NQ):           # l-quarter
                xq = xqp.tile([128, ND, 1024], bf16, tag="xq")
                for k in range(ND):
                    # interleave w1 chunk / x chunk on the sync ring so the
                    # k=0 accumulation can start after ~2 chunk arrivals
                    if q == 0:
                        nc.sync.dma_start(w1s[:, k, :], w1s_in[:, k, :])
                    nc.sync.dma_start(
                        xq[:, k, :],
                        xt_in[128 * k:128 * (k + 1), 1024 * q:1024 * (q + 1)])
                for lb in range(2):       # 512-block within quarter
                    for j in range(ND):   # dout tile
                        pm = psbig.tile([128, 512], f32, tag="pbig",
                                        name=f"p1_{q}_{lb}_{j}")
                        for k in range(ND):
                            nc.tensor.matmul(
                                pm[:],
                                w1s[:, k, 128 * j:128 * (j + 1)],
                                xq[:, k, 512 * lb:512 * (lb + 1)],
                                start=(k == 0), stop=(k == ND - 1))
                        lo = 1024 * q + 512 * lb
                        nc.scalar.activation(
                            xt[:, j, lo:lo + 512], pm[:],
                            AF.Relu, bias=b1s[:, j:j + 1], scale=1.0)
                # after each l-half completes, DMA-transpose it into xb
                # (scalar/ACT HWDGE ring, so xq loads on sync aren't blocked)
                if q in (1, 3):
                    h0 = 2048 * (q // 2)
                    for j in range(ND):
                        nc.scalar.dma_start_transpose(
                            xb[:, 16 * (q // 2):16 * (q // 2) + 16,
                               128 * j:128 * (j + 1)],
                            xt[:, j, h0:h0 + 2048])
                if q == 0:
                    # state loads on the scalar ring (consumed from NMF
                    # start; overlap phase-1 compute)
                    b2f = st.tile([128, DM], f32, tag="b2f")
                    nc.scalar.dma_start(b2f[:], b2f_in[:, :])
                    dt4_f = st.tile([128, L // 4], f32, tag="dt4f")
                    nc.scalar.dma_start(dt4_f[:], dt4_in[:, :])
                    dt4_b = st.tile([128, L // 4], bf16, tag="dt4b")
                    nc.scalar.dma_start(dt4_b[:], dt4b_in[:, :])
                    c4c_f = st.tile([128, DM // 4], f32, tag="c4cf")
                    nc.scalar.dma_start(c4c_f[:], c4c_in[:, :])
                    c4c_b = st.tile([128, DM // 4], bf16, tag="c4cb")
                    nc.scalar.dma_start(c4c_b[:], c4cb_in[:, :])
                    dnat = st.tile([128, ND, 128], bf16, tag="dnat")
                    nc.scalar.dma_start(dnat[:], dnat_in[:, :, :])

            # w2 load (reuses w1 slot; waits until phase-1 reads done)
            w2s = wpool.tile([128, ND, DM], bf16, tag="wts")
            nc.scalar.dma_start(w2s[:], w2s_in[:, :, :])

            # ---------------- phase 2: NMF steps ------------------------
            # l-tile order for DtD/DtX: m-major, so the first iterations
            # only need the dnat quarters rebuilt earliest by the previous
            # step's pipelined D-update.
            i_order = sorted(range(NL), key=lambda i: (i % 8, i // 8))
            for s in range(K_STEPS):
                # --- DtD [32,32] = sum_i dnat_i^T dnat_i
                p_dtd = pssm.tile([32, 32], f32, tag="psm", name=f"dtd{s}")
                for n, i in enumerate(i_order):
                    lhsT = dnat[:, i % 8, 32 * (i // 8):32 * (i // 8) + 32]
                    nc.tensor.matmul(p_dtd[:], lhsT, lhsT,
                                     start=(n == 0), stop=(n == NL - 1))

                # --- DtD replicated to 4 partition quarters (bf16 lhsT)
                dtd4 = wk.tile([128, 32], bf16, tag="dtd4", name=f"dtd4_{s}")
                for b in range(4):
                    nc.any.tensor_copy(dtd4[32 * b:32 * b + 32, :], p_dtd[:])

                # --- DtDC in c4c layout: diagonal 32x32 tiles
                p_dc = pssm.tile([128, 256], f32, tag="psm", name=f"dc{s}")
                for b in range(4):
                    nc.tensor.matmul(
                        p_dc[32 * b:32 * b + 32, :],
                        dtd4[32 * b:32 * b + 32, :],
                        c4c_b[32 * b:32 * b + 32, :],
                        start=True, stop=True,
                        tile_position=(32 * b, 32 * b))

                # --- DtX in c4c layout: [(b,r), d'] col-tiled 4x
                p_dtx = pssm.tile([128, 256], f32, tag="psm", name=f"dtx{s}")
                for n, i in enumerate(i_order):
                    lhsT = dnat[:, i % 8, 32 * (i // 8):32 * (i // 8) + 32]
                    for b in range(4):
                        nc.tensor.matmul(
                            p_dtx[32 * b:32 * b + 32, :], lhsT,
                            xb[:, i, 256 * b:256 * (b + 1)],
                            start=(n == 0), stop=(n == NL - 1),
                            tile_position=(0, 32 * b))

                # --- C update: C *= DtX / (DtDC + eps)
                recip_c = wk.tile([128, 256], f32, tag="rc", name=f"rc{s}")
                nc.scalar.activation(p_dc[:], p_dc[:], AF.Ln, bias=eps_c[:, 0:1])
                nc.scalar.activation(recip_c[:], p_dc[:], AF.Exp, scale=-1.0)
                nc.vector.tensor_tensor(
                    out=p_dtx[:], in0=c4c_f[:], in1=p_dtx[:], op=ALU.mult)
                nc.vector.tensor_tensor(
                    out=c4c_f[:], in0=p_dtx[:], in1=recip_c[:], op=ALU.mult)
                nc.vector.tensor_copy(c4c_b[:], c4c_f[:])

                # --- CT chunks via DMA transpose: ct[p, m, 32b+r] = C^T chunk
                ct = wk.tile([128, 2, 128], bf16, tag="ct", name=f"ct{s}")
                nc.sync.dma_start_transpose(ct[:, :, :], c4c_b[:, :])

                # --- CCt [32,32] = sum_k CT_k^T CT_k
                p_cct = pssm.tile([32, 32], f32, tag="psm", name=f"cct{s}")
                for k in range(ND):
                    lhsT = ct[:, k % 2, 32 * (k // 2):32 * (k // 2) + 32]
                    nc.tensor.matmul(p_cct[:], lhsT, lhsT,
                                     start=(k == 0), stop=(k == ND - 1))
                cct4 = wk.tile([128, 32], bf16, tag="cct4", name=f"cct4_{s}")
                for b in range(4):
                    nc.any.tensor_copy(cct4[32 * b:32 * b + 32, :], p_cct[:])

                # --- XCt in dt4 layout: [(a,r), l'] col-tiled 4x over l-quarters
                p_xct = psbig.tile([128, 1024], f32, tag="pbig", name=f"xct{s}")
                for k in range(ND):
                    lhsT = ct[:, k % 2, 32 * (k // 2):32 * (k // 2) + 32]
                    for h in range(2):
                        for a in range(4):
                            lo = 1024 * a + 512 * h
                            nc.tensor.matmul(
                                p_xct[32 * a:32 * a + 32, 512 * h:512 * (h + 1)],
                                lhsT, xt[:, k, lo:lo + 512],
                                start=(k == 0), stop=(k == ND - 1),
                                tile_position=(0, 32 * a))

                # --- DCCt in dt4 layout: diagonal tiles
                p_dcc = psbig.tile([128, 1024], f32, tag="pbig", name=f"dcc{s}")
                for h in range(2):
                    for a in range(4):
                        nc.tensor.matmul(
                            p_dcc[32 * a:32 * a + 32, 512 * h:512 * (h + 1)],
                            cct4[32 * a:32 * a + 32, :],
                            dt4_b[32 * a:32 * a + 32, 512 * h:512 * (h + 1)],
                            start=True, stop=True,
                            tile_position=(32 * a, 32 * a))

                # --- D update: D *= XCt / (DCCt + eps), split in halves so
                # ACT (Ln/Exp), DVE (muls/cast) and DMA (dnat transpose)
                # pipeline against each other.
                recip_d = wk.tile([128, 1024], f32, tag="rd", name=f"rd{s}")
                if s < K_STEPS - 1:
                    dnat = st.tile([128, ND, 128], bf16, tag="dnat")
                for h in range(4):
                    sl = slice(256 * h, 256 * (h + 1))
                    nc.scalar.activation(p_dcc[:, sl], p_dcc[:, sl],
                                         AF.Ln, bias=eps_c[:, 0:1])
                    nc.scalar.activation(recip_d[:, sl], p_dcc[:, sl],
                                         AF.Exp, scale=-1.0)
                    nc.vector.tensor_tensor(
                        out=p_xct[:, sl], in0=dt4_f[:, sl],
                        in1=p_xct[:, sl], op=ALU.mult)
                    nc.vector.tensor_tensor(
                        out=dt4_f[:, sl], in0=p_xct[:, sl],
                        in1=recip_d[:, sl], op=ALU.mult)
                    nc.vector.tensor_copy(dt4_b[:, sl], dt4_f[:, sl])
                    # rebuild D natural (lhsT for next step's DtX/DtD)
                    if s < K_STEPS - 1:
                        nc.sync.dma_start_transpose(
                            dnat[:, 2 * h:2 * h + 2, :], dt4_b[:, sl])

            # ---------------- phase 3: out = D @ (C @ W2) + b2 ----------
            p_c2 = psbig.tile([32, 1024], f32, tag="pbig", name="c2")
            for k in range(ND):
                lhsT = ct[:, k % 2, 32 * (k // 2):32 * (k // 2) + 32]
                for h in range(2):
                    nc.tensor.matmul(
                        p_c2[:, 512 * h:512 * (h + 1)], lhsT,
                        w2s[:, k, 512 * h:512 * (h + 1)],
                        start=(k == 0), stop=(k == ND - 1))
            c2b4 = st.tile([128, DM], bf16, tag="c2b4")
            for b in range(4):
                nc.any.tensor_copy(c2b4[32 * b:32 * b + 32, :], p_c2[:])

            # out tiles: l-tile i = 8a + step handled by K=32 row group a;
            # 4 tiles (one per row group, concurrent on PE) are batched
            # into one 2 MB output DMA. ot slots reuse the xq pool.
            out_v = out.rearrange("(a s p) d -> p a s d", a=4, p=128)
            for step in range(8):
                for g in range(2):        # row-group pair (a = 2g, 2g+1)
                    ot = xqp.tile([128, 2, DM], f32, tag="xq",
                                  name=f"ot{step}_{g}")
                    for u in range(2):
                        a = 2 * g + u
                        po = psbig.tile([128, 1024], f32, tag="pbig",
                                        name=f"po{step}_{a}")
                        for h in range(2):
                            nc.tensor.matmul(
                                po[:, 512 * h:512 * (h + 1)],
                                dt4_b[32 * a:32 * a + 32,
                                      128 * step:128 * (step + 1)],
                                c2b4[32 * a:32 * a + 32,
                                     512 * h:512 * (h + 1)],
                                start=True, stop=True,
                                tile_position=(32 * a, 0))
                        nc.vector.tensor_tensor(
                            out=ot[:, u, :], in0=po[:], in1=b2f[:],
                            op=ALU.add)
                    nc.sync.dma_start(
                        out_v[:, 2 * g:2 * g + 2, step, :], ot[:, :, :])

    nc.finalize()
    return nc


def prep_in_maps(inputs):
    """Host-side layout prep (transpose/cast/tile only - no FLOPs)."""
    import ml_dtypes
    bf16 = ml_dtypes.bfloat16

    x = np.asarray(inputs["input_tensor"], np.float32)       # [L, B, DM]
    w1 = np.asarray(inputs["W1"], np.float32)
    w2 = np.asarray(inputs["W2"], np.float32)
    b1 = np.asarray(inputs["b1"], np.float32)
    b2 = np.asarray(inputs["b2"], np.float32)
    d0 = np.asarray(inputs["D_init"], np.float32)            # [L, R]
    c0 = np.asarray(inputs["C_init"], np.float32)            # [R, DM]

    def chunk_major(w):  # [DM, DM] -> [128, ND, DM]
        return np.ascontiguousarray(
            w.reshape(ND, 128, DM).transpose(1, 0, 2).astype(bf16))

    # dt4[(a,r), l'] = D^T[r, 1024a + l']
    dt4 = np.ascontiguousarray(
        d0.reshape(4, 1024, R).transpose(0, 2, 1).reshape(128, 1024))
    # dnat0[p, m, 32a + r] = D[128(8a + m) + p, r]
    dnat0 = np.ascontiguousarray(
        d0.reshape(4, 8, 128, R).transpose(2, 1, 0, 3).reshape(128, 8, 128)
        .astype(bf16))
    # c4c[(b,r), d'] = C[r, 256b + d']
    c4c = np.ascontiguousarray(
        c0.reshape(R, 4, 256).transpose(1, 0, 2).reshape(128, 256))

    shared = {
        "w1s": chunk_major(w1),
        "w2s": chunk_major(w2),
        "b1s": np.ascontiguousarray(b1.reshape(ND, 128).T),
        "b2f": np.ascontiguousarray(np.tile(b2.reshape(1, DM), (128, 1))),
        "dt4": dt4,
        "dt4b": np.ascontiguousarray(dt4.astype(bf16)),
        "dnat0": dnat0,
        "c4c": c4c,
        "c4cb": np.ascontiguousarray(c4c.astype(bf16)),
    }
    in_maps = []
    for b in range(B):
        xt_b = np.ascontiguousarray(x[:, b, :].T.astype(bf16))  # [DM, L]
        in_maps.append({"xt_in": xt_b, **shared})
    return in_maps


_NC_CACHE = None


def _kernel_numpy(inputs):
    """Correct host fallback (only if the Bass path fails)."""
    X0 = np.transpose(np.asarray(inputs["input_tensor"], np.float32), (1, 0, 2))
    W1 = np.asarray(inputs["W1"], np.float32); b1 = np.asarray(inputs["b1"], np.float32)
    W2 = np.asarray(inputs["W2"], np.float32); b2 = np.asarray(inputs["b2"], np.float32)
    outs = []
    for b in range(B):
        X = np.maximum(X0[b] @ W1 + b1, 0.0)
        D = np.asarray(inputs["D_init"], np.float32).copy()
        C = np.asarray(inputs["C_init"], np.float32).copy()
        for _ in range(K_STEPS):
            C = C * (D.T @ X) / ((D.T @ D) @ C + EPS)
            D = D * (X @ C.T) / (D @ (C @ C.T) + EPS)
        outs.append((D @ C) @ W2 + b2)
    return np.stack(outs, axis=0).transpose(1, 0, 2).astype(np.float32)


def kernel(**inputs) -> np.ndarray:
    global _NC_CACHE
    try:
        from concourse.bass_utils import run_bass_kernel_spmd

        if _NC_CACHE is None:
            _NC_CACHE = build_nc()
        in_maps = prep_in_maps(inputs)
        res = run_bass_kernel_spmd(_NC_CACHE, in_maps, core_ids=list(range(B)))
        outs = [res.results[b]["out"] for b in range(B)]
        return np.stack(outs, axis=1).astype(np.float32)  # [L, B, DM]
    except Exception as e:
        print(f"kernel: Bass path failed ({type(e).__name__}: {e}); "
              f"falling back to numpy", file=sys.stderr)
        return _kernel_numpy(inputs)


# revision 19
# speedup vs baseline: 1.0936x; 1.0936x over previous
"""Trainium2 Bass kernel for nn_KrabbyPatty: batched NMF with MLP bread.

Per-core program (pure data parallel, one batch element per core):
  X  = relu(Xin @ W1 + b1)                  # [4096, 1024]
  D, C = D_init, C_init
  repeat 6x:
    C = C * (D^T X) / (D^T D C + eps)
    D = D * (X C^T) / (D C C^T + eps)
  out = D @ (C @ W2) + b2

Key layout/engine choices:
  - Host prep (layout only): Xin transposed + cast bf16; weights cast bf16
    and chunk-major; D/C initial states pre-arranged in device layouts.
  - XT = X^T [dout-part, l] computed by dense bf16 matmuls (PE), relu+bias
    fused on ScalarE during PSUM->SBUF eviction.
  - XB = X natural built from XT with DMA x-bar transposes (bf16,
    SBUF->SBUF) - zero PE/DVE cost.
  - D state in "dt4" layout [128=(4 l-quarters x 32r), 1024 l'] = folded
    D^T; C state in "c4c" layout [128=(4 d-quarters x 32r), 256 d'].
    All elementwise updates run on full 128 partitions.
  - D^T X and X C^T use 4x column-tiled matmuls (tile_position=(0,32b)):
    four concurrent rhs streams -> ~4x fewer PE cycles per X pass, and
    the quarter-partials land directly in the dt4/c4c layouts (the
    "reduction" is PSUM accumulation - no cross-partition sums needed).
  - D C C^T and DtD C use diagonal 32x32 tiles (tile_position=(32a,32a)).
  - Division via Ln/Exp on ScalarE (nc.scalar Reciprocal is banned for
    accuracy); numerator products on DVE with PSUM in-place reuse.
  - dnat (D natural, lhsT for D^T X) and CT (C^T chunks, lhsT for C X^T)
    rebuilt each step with one small DMA transpose each.
  - Final: C2 = C@W2 (8k accum), out tiles = D@C2 with K=32 row-tiled
    matmuls interleaved across the 4 row groups; b2 added on DVE during
    PSUM eviction, hidden behind the output DMA.
"""

import sys
import numpy as np

L, B, DM, R, K_STEPS = 4096, 8, 1024, 32, 6
EPS = 1e-9
NL = L // 128   # 32 l-tiles
ND = DM // 128  # 8 d-chunks
NQ = 4          # l-quarters (1024 each)


def build_nc():
    import concourse.bacc as bacc
    import concourse.mybir as mybir
    import concourse.tile as tile

    f32 = mybir.dt.float32
    bf16 = mybir.dt.bfloat16
    AF = mybir.ActivationFunctionType
    ALU = mybir.AluOpType

    class _Bacc(bacc.Bacc):
        """Pin all activations to the one act-func table that contains
        every function this kernel uses (relu/ln/exp/copy/identity).

        The default per-instruction set choice alternates between the
        `natural_log` and `exp_and_others` tables, inserting a ~1.3us
        ACT_TABLE_LOAD on every Ln<->Exp<->Copy transition (4+ per NMF
        step).  `natural_log_exp_and_others` holds all of them at once;
        hiding my funcs from the other sets (list order/length kept, so
        act_func_set_id indices still match act_info.json) makes the
        fixpoint hoist a single load for the whole kernel.
        """

        _KEEP = "natural_log_exp_and_others"

        def insert_act_table_loads(self):
            from concourse.hw_specs import get_activation_tables
            import bass_rust as _br

            has_activation = any(
                isinstance(i, mybir.InstActivation)
                for b in self.main_func.blocks
                for i in b.instructions
            )
            if not has_activation:
                return
            used = {
                i.func
                for b in self.main_func.blocks
                for i in b.instructions
                if isinstance(i, mybir.InstActivation)
            }
            tables = list(get_activation_tables(self.m.arch).items())
            keep_funcs = dict(tables)[self._KEEP]
            if used <= keep_funcs:
                tables = [
                    (name, funcs if name == self._KEEP else funcs - used)
                    for name, funcs in tables
                ]
            _br.insert_act_table_loads(self, tables)

    nc = _Bacc()
    xt_in = nc.dram_tensor("xt_in", [DM, L], bf16, kind="ExternalInput")
    w1s_in = nc.dram_tensor("w1s", [128, ND, DM], bf16, kind="ExternalInput")
    w2s_in = nc.dram_tensor("w2s", [128, ND, DM], bf16, kind="ExternalInput")
    b1s_in = nc.dram_tensor("b1s", [128, ND], f32, kind="ExternalInput")
    b2f_in = nc.dram_tensor("b2f", [128, DM], f32, kind="ExternalInput")
    dt4_in = nc.dram_tensor("dt4", [128, L // 4], f32, kind="ExternalInput")
    dt4b_in = nc.dram_tensor("dt4b", [128, L // 4], bf16, kind="ExternalInput")
    dnat_in = nc.dram_tensor("dnat0", [128, ND, 128], bf16, kind="ExternalInput")
    c4c_in = nc.dram_tensor("c4c", [128, DM // 4], f32, kind="ExternalInput")
    c4cb_in = nc.dram_tensor("c4cb", [128, DM // 4], bf16, kind="ExternalInput")
    out = nc.dram_tensor("out", [L, DM], f32, kind="ExternalOutput")

    with tile.TileContext(nc) as tc:
        with (
            tc.tile_pool(name="bigsb", bufs=1) as bigsb,    # xt / xb
            tc.tile_pool(name="wpool", bufs=1) as wpool,    # w1s then w2s
            tc.tile_pool(name="xq", bufs=2) as xqp,         # Xin^T quarter staging
            tc.tile_pool(name="state", bufs=1) as st,       # D/C state + consts
            tc.tile_pool(name="work", bufs=1) as wk,        # per-step recompute
            tc.tile_pool(name="big", bufs=2, space="PSUM") as psbig,    # [128,1024]
            tc.tile_pool(name="small", bufs=4, space="PSUM") as pssm,   # [128,256]
        ):
            # ---------------- constants / initial state ----------------
            b1s = st.tile([128, ND], f32, tag="b1s")
            nc.sync.dma_start(b1s[:], b1s_in[:, :])
            eps_c = st.tile([128, 1], f32, tag="epsc")
            nc.vector.memset(eps_c[:], EPS)

            # ---------------- phase 1: XT = relu(W1^T Xin^T + b1) ------
            # xt[p, j, l] = X[l, 128j + p];  xb[p, i, d] = X[128i + p, d]
            xt = bigsb.tile([128, ND, L], bf16, tag="xt")
            xb = bigsb.tile([128, NL, DM], bf16, tag="xb")
            w1s = wpool.tile([128, ND, DM], bf16, tag="wts")

            for q in range(NQ):           # l-quarter
                xq = xqp.tile([128, ND, 1024], bf16, tag="xq")
                for k in range(ND):
                    # interleave w1 chunk / x chunk on the sync ring so the
                    # k=0 accumulation can start after ~2 chunk arrivals
                    if q == 0:
                        nc.sync.dma_start(w1s[:, k, :], w1s_in[:, k, :])
                    nc.sync.dma_start(
                        xq[:, k, :],
                        xt_in[128 * k:128 * (k + 1), 1024 * q:1024 * (q + 1)])
                for lb in range(2):       # 512-block within quarter
                    for j in range(ND):   # dout tile
                        pm = psbig.tile([128, 512], f32, tag="pbig",
                                        name=f"p1_{q}_{lb}_{j}")
                        for k in range(ND):
                            nc.tensor.matmul(
                                pm[:],
                                w1s[:, k, 128 * j:128 * (j + 1)],
                                xq[:, k, 512 * lb:512 * (lb + 1)],
                                start=(k == 0), stop=(k == ND - 1))
                        lo = 1024 * q + 512 * lb
                        nc.scalar.activation(
                            xt[:, j, lo:lo + 512], pm[:],
                            AF.Relu, bias=b1s[:, j:j + 1], scale=1.0)
                # after each l-half completes, DMA-transpose it into xb
                # (scalar/ACT HWDGE ring, so xq loads on sync aren't blocked)
                if q in (1, 3):
                    h0 = 2048 * (q // 2)
                    for j in range(ND):
                        nc.sync.dma_start_transpose(
                            xb[:, 16 * (q // 2):16 * (q // 2) + 16,
                               128 * j:128 * (j + 1)],
                            xt[:, j, h0:h0 + 2048])
                if q == 0:
                    # state loads on the gpsimd (SWDGE) ring: off both the
                    # sync ring (xq loads) and the ACT stream (relu)
                    b2f = st.tile([128, DM], f32, tag="b2f")
                    nc.gpsimd.dma_start(b2f[:], b2f_in[:, :])
                    dt4_f = st.tile([128, L // 4], f32, tag="dt4f")
                    nc.gpsimd.dma_start(dt4_f[:], dt4_in[:, :])
                    dt4_b = st.tile([128, L // 4], bf16, tag="dt4b")
                    nc.gpsimd.dma_start(dt4_b[:], dt4b_in[:, :])
                    c4c_f = st.tile([128, DM // 4], f32, tag="c4cf")
                    nc.gpsimd.dma_start(c4c_f[:], c4c_in[:, :])
                    c4c_b = st.tile([128, DM // 4], bf16, tag="c4cb")
                    nc.gpsimd.dma_start(c4c_b[:], c4cb_in[:, :])
                    dnat = st.tile([128, ND, 128], bf16, tag="dnat")
                    nc.gpsimd.dma_start(dnat[:], dnat_in[:, :, :])

            # w2 load (reuses w1 slot; waits until phase-1 reads done)
            w2s = wpool.tile([128, ND, DM], bf16, tag="wts")
            nc.gpsimd.dma_start(w2s[:], w2s_in[:, :, :])

            # ---------------- phase 2: NMF steps ------------------------
            # l-tile order for DtD/DtX: m-major, so the first iterations
            # only need the dnat quarters rebuilt earliest by the previous
            # step's pipelined D-update.
            i_order = sorted(range(NL), key=lambda i: (i % 8, i // 8))
            for s in range(K_STEPS):
                # --- DtD [32,32] = sum_i dnat_i^T dnat_i
                p_dtd = pssm.tile([32, 32], f32, tag="psm", name=f"dtd{s}")
                for n, i in enumerate(i_order):
                    lhsT = dnat[:, i % 8, 32 * (i // 8):32 * (i // 8) + 32]
                    nc.tensor.matmul(p_dtd[:], lhsT, lhsT,
                                     start=(n == 0), stop=(n == NL - 1))

                # --- DtD replicated to 4 partition quarters (bf16 lhsT)
                dtd4 = wk.tile([128, 32], bf16, tag="dtd4", name=f"dtd4_{s}")
                for b in range(4):
                    nc.any.tensor_copy(dtd4[32 * b:32 * b + 32, :], p_dtd[:])

                # --- DtDC in c4c layout: diagonal 32x32 tiles
                p_dc = pssm.tile([128, 256], f32, tag="psm", name=f"dc{s}")
                for b in range(4):
                    nc.tensor.matmul(
                        p_dc[32 * b:32 * b + 32, :],
                        dtd4[32 * b:32 * b + 32, :],
                        c4c_b[32 * b:32 * b + 32, :],
                        start=True, stop=True,
                        tile_position=(32 * b, 32 * b))

                # --- DtX in c4c layout: [(b,r), d'] col-tiled 4x
                p_dtx = pssm.tile([128, 256], f32, tag="psm", name=f"dtx{s}")
                for n, i in enumerate(i_order):
                    lhsT = dnat[:, i % 8, 32 * (i // 8):32 * (i // 8) + 32]
                    for b in range(4):
                        nc.tensor.matmul(
                            p_dtx[32 * b:32 * b + 32, :], lhsT,
                            xb[:, i, 256 * b:256 * (b + 1)],
                            start=(n == 0), stop=(n == NL - 1),
                            tile_position=(0, 32 * b))

                # --- C update: C *= DtX / (DtDC + eps)
                recip_c = wk.tile([128, 256], f32, tag="rc", name=f"rc{s}")
                nc.scalar.activation(p_dc[:], p_dc[:], AF.Ln, bias=eps_c[:, 0:1])
                nc.scalar.activation(recip_c[:], p_dc[:], AF.Exp, scale=-1.0)
                nc.vector.tensor_tensor(
                    out=p_dtx[:], in0=c4c_f[:], in1=p_dtx[:], op=ALU.mult)
                nc.vector.tensor_tensor(
                    out=c4c_f[:], in0=p_dtx[:], in1=recip_c[:], op=ALU.mult)
                nc.vector.tensor_copy(c4c_b[:], c4c_f[:])

                # --- CT chunks via DMA transpose: ct[p, m, 32b+r] = C^T chunk
                ct = wk.tile([128, 2, 128], bf16, tag="ct", name=f"ct{s}")
                nc.sync.dma_start_transpose(ct[:, :, :], c4c_b[:, :])

                # --- CCt [32,32] = sum_k CT_k^T CT_k
                p_cct = pssm.tile([32, 32], f32, tag="psm", name=f"cct{s}")
                for k in range(ND):
                    lhsT = ct[:, k % 2, 32 * (k // 2):32 * (k // 2) + 32]
                    nc.tensor.matmul(p_cct[:], lhsT, lhsT,
                                     start=(k == 0), stop=(k == ND - 1))
                cct4 = wk.tile([128, 32], bf16, tag="cct4", name=f"cct4_{s}")
                for b in range(4):
                    nc.any.tensor_copy(cct4[32 * b:32 * b + 32, :], p_cct[:])

                # --- XCt in dt4 layout: [(a,r), l'] col-tiled 4x over l-quarters
                p_xct = psbig.tile([128, 1024], f32, tag="pbig", name=f"xct{s}")
                for k in range(ND):
                    lhsT = ct[:, k % 2, 32 * (k // 2):32 * (k // 2) + 32]
                    for h in range(2):
                        for a in range(4):
                            lo = 1024 * a + 512 * h
                            nc.tensor.matmul(
                                p_xct[32 * a:32 * a + 32, 512 * h:512 * (h + 1)],
                                lhsT, xt[:, k, lo:lo + 512],
                                start=(k == 0), stop=(k == ND - 1),
                                tile_position=(0, 32 * a))

                # --- DCCt in dt4 layout: diagonal tiles
                p_dcc = psbig.tile([128, 1024], f32, tag="pbig", name=f"dcc{s}")
                for h in range(2):
                    for a in range(4):
                        nc.tensor.matmul(
                            p_dcc[32 * a:32 * a + 32, 512 * h:512 * (h + 1)],
                            cct4[32 * a:32 * a + 32, :],
                            dt4_b[32 * a:32 * a + 32, 512 * h:512 * (h + 1)],
                            start=True, stop=True,
                            tile_position=(32 * a, 32 * a))

                # --- D update: D *= XCt / (DCCt + eps), split in halves so
                # ACT (Ln/Exp), DVE (muls/cast) and DMA (dnat transpose)
                # pipeline against each other.
                recip_d = wk.tile([128, 1024], f32, tag="rd", name=f"rd{s}")
                if s < K_STEPS - 1:
                    dnat = st.tile([128, ND, 128], bf16, tag="dnat")
                for h in range(2):
                    sl = slice(512 * h, 512 * (h + 1))
                    nc.scalar.activation(p_dcc[:, sl], p_dcc[:, sl],
                                         AF.Ln, bias=eps_c[:, 0:1])
                    nc.scalar.activation(recip_d[:, sl], p_dcc[:, sl],
                                         AF.Exp, scale=-1.0)
                    nc.vector.tensor_tensor(
                        out=p_xct[:, sl], in0=dt4_f[:, sl],
                        in1=p_xct[:, sl], op=ALU.mult)
                    nc.vector.tensor_tensor(
                        out=dt4_f[:, sl], in0=p_xct[:, sl],
                        in1=recip_d[:, sl], op=ALU.mult)
                    nc.vector.tensor_copy(dt4_b[:, sl], dt4_f[:, sl])
                    # rebuild D natural (lhsT for next step's DtX/DtD)
                    if s < K_STEPS - 1:
                        nc.sync.dma_start_transpose(
                            dnat[:, 4 * h:4 * h + 4, :], dt4_b[:, sl])

            # ---------------- phase 3: out = D @ (C @ W2) + b2 ----------
            p_c2 = psbig.tile([32, 1024], f32, tag="pbig", name="c2")
            for k in range(ND):
                lhsT = ct[:, k % 2, 32 * (k // 2):32 * (k // 2) + 32]
                for h in range(2):
                    nc.tensor.matmul(
                        p_c2[:, 512 * h:512 * (h + 1)], lhsT,
                        w2s[:, k, 512 * h:512 * (h + 1)],
                        start=(k == 0), stop=(k == ND - 1))
            c2b4 = st.tile([128, DM], bf16, tag="c2b4")
            for b in range(4):
                nc.any.tensor_copy(c2b4[32 * b:32 * b + 32, :], p_c2[:])

            # out tiles: l-tile i = 8a + step handled by K=32 row group a;
            # 4 tiles (one per row group, concurrent on PE) are batched
            # into one 2 MB output DMA. ot slots reuse the xq pool.
            out_v = out.rearrange("(a s p) d -> p a s d", a=4, p=128)
            for step in range(8):
                ot = xqp.tile([128, 4, DM], f32, tag="xq", name=f"ot{step}")
                for a in range(4):
                    po = psbig.tile([128, 1024], f32, tag="pbig",
                                    name=f"po{step}_{a}")
                    for h in range(2):
                        nc.tensor.matmul(
                            po[:, 512 * h:512 * (h + 1)],
                            dt4_b[32 * a:32 * a + 32,
                                  128 * step:128 * (step + 1)],
                            c2b4[32 * a:32 * a + 32, 512 * h:512 * (h + 1)],
                            start=True, stop=True,
                            tile_position=(32 * a, 0))
                    nc.vector.tensor_tensor(
                        out=ot[:, a, :], in0=po[:], in1=b2f[:], op=ALU.add)
                nc.sync.dma_start(out_v[:, :, step, :], ot[:, :, :])

    nc.finalize()
    return nc


def prep_in_maps(inputs):
    """Host-side layout prep (transpose/cast/tile only - no FLOPs)."""
    import ml_dtypes
    bf16 = ml_dtypes.bfloat16

    x = np.asarray(inputs["input_tensor"], np.float32)       # [L, B, DM]
    w1 = np.asarray(inputs["W1"], np.float32)
    w2 = np.asarray(inputs["W2"], np.float32)
    b1 = np.asarray(inputs["b1"], np.float32)
    b2 = np.asarray(inputs["b2"], np.float32)
    d0 = np.asarray(inputs["D_init"], np.float32)            # [L, R]
    c0 = np.asarray(inputs["C_init"], np.float32)            # [R, DM]

    def chunk_major(w):  # [DM, DM] -> [128, ND, DM]
        return np.ascontiguousarray(
            w.reshape(ND, 128, DM).transpose(1, 0, 2).astype(bf16))

    # dt4[(a,r), l'] = D^T[r, 1024a + l']
    dt4 = np.ascontiguousarray(
        d0.reshape(4, 1024, R).transpose(0, 2, 1).reshape(128, 1024))
    # dnat0[p, m, 32a + r] = D[128(8a + m) + p, r]
    dnat0 = np.ascontiguousarray(
        d0.reshape(4, 8, 128, R).transpose(2, 1, 0, 3).reshape(128, 8, 128)
        .astype(bf16))
    # c4c[(b,r), d'] = C[r, 256b + d']
    c4c = np.ascontiguousarray(
        c0.reshape(R, 4, 256).transpose(1, 0, 2).reshape(128, 256))

    shared = {
        "w1s": chunk_major(w1),
        "w2s": chunk_major(w2),
        "b1s": np.ascontiguousarray(b1.reshape(ND, 128).T),
        "b2f": np.ascontiguousarray(np.tile(b2.reshape(1, DM), (128, 1))),
        "dt4": dt4,
        "dt4b": np.ascontiguousarray(dt4.astype(bf16)),
        "dnat0": dnat0,
        "c4c": c4c,
        "c4cb": np.ascontiguousarray(c4c.astype(bf16)),
    }
    in_maps = []
    for b in range(B):
        xt_b = np.ascontiguousarray(x[:, b, :].T.astype(bf16))  # [DM, L]
        in_maps.append({"xt_in": xt_b, **shared})
    return in_maps


_NC_CACHE = None


def _kernel_numpy(inputs):
    """Correct host fallback (only if the Bass path fails)."""
    X0 = np.transpose(np.asarray(inputs["input_tensor"], np.float32), (1, 0, 2))
    W1 = np.asarray(inputs["W1"], np.float32); b1 = np.asarray(inputs["b1"], np.float32)
    W2 = np.asarray(inputs["W2"], np.float32); b2 = np.asarray(inputs["b2"], np.float32)
    outs = []
    for b in range(B):
        X = np.maximum(X0[b] @ W1 + b1, 0.0)
        D = np.asarray(inputs["D_init"], np.float32).copy()
        C = np.asarray(inputs["C_init"], np.float32).copy()
        for _ in range(K_STEPS):
            C = C * (D.T @ X) / ((D.T @ D) @ C + EPS)
            D = D * (X @ C.T) / (D @ (C @ C.T) + EPS)
        outs.append((D @ C) @ W2 + b2)
    return np.stack(outs, axis=0).transpose(1, 0, 2).astype(np.float32)


def kernel(**inputs) -> np.ndarray:
    global _NC_CACHE
    try:
        from concourse.bass_utils import run_bass_kernel_spmd

        if _NC_CACHE is None:
            _NC_CACHE = build_nc()
        in_maps = prep_in_maps(inputs)
        res = run_bass_kernel_spmd(_NC_CACHE, in_maps, core_ids=list(range(B)))
        outs = [res.results[b]["out"] for b in range(B)]
        return np.stack(outs, axis=1).astype(np.float32)  # [L, B, DM]
    except Exception as e:
        print(f"kernel: Bass path failed ({type(e).__name__}: {e}); "
              f"falling back to numpy", file=sys.stderr)
        return _kernel_numpy(inputs)
